# revision 1
# baseline (speedup 1.0000x reference)
"""Seq2seq LSTM (CoordinatePredictionModel) Trainium2 Bass kernel.

Model: 200-step LSTM encoder over [T=200, B=4096, IN=4], then 30-step
autoregressive LSTM decoder with output projection -> [30, B, OUT=4].

Sharding: pure data-parallel over batch. B=4096 -> 512 per core x 8 cores,
no collectives. Each core runs the full 230-step recurrence.

Layout ("hidden on partitions"): per-step state lives transposed in one
SBUF tile s = [K=105, 512]: rows 0..99 = h^T, rows 100..103 = x^T (encoder
input; engine-unused in decoder), row 104 = ones. Gate pre-activations for
chunk g come from one matmul
  psum[:, g*512:(g+1)*512] = W_g^T.T @ s       (K=105 contraction)
with W_g^T = [W_hh_g.T ; W_ih_g.T ; (b_ih+b_hh)_g] stacked on partitions, so
the input projection and both biases ride along in K. Gates are reordered
[i, f, o, g] so one sigmoid covers all three sigmoid chunks.

Decoder feedback y_prev = W_y h + b_y is folded into the recurrence:
  W_ih_dec @ y_prev + W_hh_dec @ h = (W_ih_dec W_y + W_hh_dec) @ h + W_ih_dec b_y
(valid from the second decoder step; the first uses y_prev = 0), so y is
only ever computed PSUM -> DRAM and never re-enters SBUF state.

Only DMA may write non-32-aligned partition bases; all engine writes here
start at partition 0 (x and ones rows are DMA-written).
"""

import numpy as np

import concourse.bass as bass
import concourse.mybir as mybir
from concourse import bacc
from concourse.tile import TileContext
from concourse.tile_rust import add_dep_helper
from concourse.bass_utils import run_bass_kernel_spmd

T, B, IN, OUT, H = 200, 4096, 4, 4, 100
DEC = 30
NCORES = 8
BS = B // NCORES          # 512 batch rows per core
K = H + IN + 1            # 105 = h + x + ones
F32 = mybir.dt.float32

# matmul compute dtype: "f32" (exact, 4 cyc/row), "f32r" (tf32-ish, 1 cyc/row)
MM_DT = "f32r"

# gate reorder: pytorch order [i, f, g, o] -> ours [i, f, o, g]
_PERM = np.concatenate([np.arange(0, 100), np.arange(100, 200),
                        np.arange(300, 400), np.arange(200, 300)])


def _pack_weights(W_hh, W_ih, bias):
    """[K=105, 4H] stacked lhsT with gate columns reordered [i,f,o,g]."""
    Wk = np.zeros((K, 4 * H), np.float64)
    Wk[0:H, :] = W_hh.T[:, _PERM]
    if W_ih is not None:
        Wk[H:H + W_ih.shape[1], :] = W_ih.T[:, _PERM]
    Wk[K - 1, :] = bias[_PERM]
    return Wk.astype(np.float32)


# dtype used for matmul operands (DRAM + SBUF); fp32r is fp32 bits in
# memory, reduced-precision (1 cycle/row) on the PE
MDT = mybir.dt.float32r if MM_DT == "f32r" else F32

# elementwise dtype for gate tensors (sig/tg/t1/t2/c/tct)
EWDT = F32


def _repeat_steps(repeat):
    """Step indices; extra repeats of the whole recurrence for timing only."""
    steps = list(range(T + DEC))
    return steps * repeat


LABELS = {}


def _lab(ret, name):
    try:
        LABELS[ret.ins.name] = name
    except Exception:
        pass
    return ret


def _build_program(repeat=1, nh=2, loop_n=None):
    """nh = number of independent batch half-chains interleaved per step."""
    nc = bacc.Bacc("TRN2", debug=False, num_devices=NCORES)

    x_d = nc.dram_tensor("x", (T, IN, BS), MDT, kind="ExternalInput").ap()
    encw_d = nc.dram_tensor("encw", (K, 4 * H), MDT, kind="ExternalInput").ap()
    dec0w_d = nc.dram_tensor("dec0w", (K, 4 * H), MDT, kind="ExternalInput").ap()
    decfw_d = nc.dram_tensor("decfw", (K, 4 * H), MDT, kind="ExternalInput").ap()
    wy_d = nc.dram_tensor("wy", (K, OUT), MDT, kind="ExternalInput").ap()
    ones_d = nc.dram_tensor("ones", (1, BS), MDT, kind="ExternalInput").ap()
    y_d = nc.dram_tensor("y", (DEC, OUT, BS), F32, kind="ExternalOutput").ap()

    AF = mybir.ActivationFunctionType
    HB = BS // nh

    with TileContext(nc) as tc:
        with (
            tc.tile_pool(name="const", bufs=1) as constp,
            tc.tile_pool(name="state", bufs=1) as statep,
            tc.tile_pool(name="work", bufs=3) as work,
            tc.tile_pool(name="psum", bufs=2, space="PSUM") as psump,
        ):
            encw = constp.tile([K, 4 * H], MDT, tag="encw")
            dec0w = constp.tile([K, 4 * H], MDT, tag="dec0w")
            decfw = constp.tile([K, 4 * H], MDT, tag="decfw")
            wy = constp.tile([K, OUT], MDT, tag="wy")
            nc.sync.dma_start(encw[:], encw_d[:])
            nc.sync.dma_start(dec0w[:], dec0w_d[:])
            nc.sync.dma_start(decfw[:], decfw_d[:])
            nc.sync.dma_start(wy[:], wy_d[:])

            sz = [[statep.tile([K, HB], MDT, tag=f"s{p}{z}", name=f"s{p}{z}")
                   for p in range(2)] for z in range(nh)]
            cz = [statep.tile([H, HB], EWDT, tag=f"c{z}", name=f"c{z}")
                  for z in range(nh)]

            for z in range(nh):
                hb = slice(z * HB, (z + 1) * HB)
                nc.gpsimd.memset(sz[z][0][0:H, :].bitcast(mybir.dt.uint32), 0)
                nc.gpsimd.memset(cz[z][:], 0.0)
                nc.sync.dma_start(sz[z][0][K - 1:K, :], ones_d[0:1, hb])
                nc.sync.dma_start(sz[z][1][K - 1:K, :], ones_d[0:1, hb])
                nc.sync.dma_start(sz[z][0][H:H + IN, :], x_d[0, :, hb])

            import contextlib
            loop_ctx = (tc.For_i(0, loop_n, 1) if loop_n is not None
                        else contextlib.nullcontext())

            def emit_P1(z, t):
                """Matmuls + tanh(g) for chain z, step t. Returns (pt, tg).

                PSUM layout is 3 banks: [i f o] in banks 0-1, g alone in
                bank 2 (offset 4*HB), so tanh(g) only waits on mm_g (issued
                first) and sigma only on mm_i/f/o."""
                W = encw if t < T else (dec0w if t == T else decfw)
                prev = sz[z][t % 2]
                pt = psump.tile([H, 4 * HB], F32, tag=f"pt{z}", name=f"pt{z}")
                _lab(nc.tensor.matmul(pt[:, 2 * HB:3 * HB], W[:, 2 * H:3 * H],
                                 prev[:], start=True, stop=True), f"mm_o z{z} t{t}")
                _lab(nc.tensor.matmul(pt[:, 3 * HB:4 * HB], W[:, 3 * H:4 * H],
                                 prev[:], start=True, stop=True), f"mm_g z{z} t{t}")
                _lab(nc.tensor.matmul(pt[:, 0:HB], W[:, 0:H], prev[:],
                                 start=True, stop=True), f"mm_i z{z} t{t}")
                _lab(nc.tensor.matmul(pt[:, HB:2 * HB], W[:, H:2 * H], prev[:],
                                 start=True, stop=True), f"mm_f z{z} t{t}")
                tg = work.tile([H, HB], EWDT, tag=f"tg{z}", name=f"tg{z}")
                _lab(nc.scalar.activation(tg[:], pt[:, 3 * HB:4 * HB], AF.Tanh), f"tg z{z} t{t}")
                return pt, tg

            def emit_sigma(z, pt):
                sig = work.tile([H, 3 * HB], EWDT, tag=f"sig{z}", name=f"sig{z}")
                _lab(nc.scalar.activation(sig[:], pt[:, 0:3 * HB], AF.Sigmoid), f"sigma z{z}")
                return sig

            def emit_P2(z, t, pt, tg, sig):
                """Cell-state tail for chain z, step t."""
                c = cz[z]
                t1 = work.tile([H, HB], EWDT, tag=f"t1{z}", name=f"t1{z}")
                _lab(nc.vector.tensor_mul(t1[:], sig[:, 0:HB], tg[:]), f"t1 z{z} t{t}")
                t2 = work.tile([H, HB], EWDT, tag=f"t2{z}", name=f"t2{z}")
                _lab(nc.gpsimd.tensor_mul(t2[:], sig[:, HB:2 * HB], c[:]), f"t2 z{z} t{t}")
                _lab(nc.vector.tensor_add(c[:], t1[:], t2[:]), f"add z{z} t{t}")
                tct = work.tile([H, HB], EWDT, tag=f"tct{z}", name=f"tct{z}")
                with tc.high_priority():
                    _lab(nc.scalar.activation(tct[:], c[:], AF.Tanh), f"tanc z{z} t{t}")
                    nxt = sz[z][(t + 1) % 2]
                    _lab(nc.vector.tensor_mul(nxt[0:H, :], sig[:, 2 * HB:3 * HB],
                                         tct[:]), f"h z{z} t{t}")
                hb = slice(z * HB, (z + 1) * HB)
                if t < T - 1:
                    nc.sync.dma_start(nxt[H:H + IN, :], x_d[t + 1, :, hb])
                elif t == T - 1:
                    nc.vector.memset(c[:], 0.0)
                else:
                    d = t - T
                    yp = pt[0:OUT, 3 * HB:4 * HB]
                    nc.tensor.matmul(yp, wy[:], nxt[:], start=True, stop=True)
                    yo = work.tile([OUT, HB], F32, tag=f"yo{z}", name=f"yo{z}")
                    nc.vector.tensor_copy(yo[:], yp)
                    nc.sync.dma_start(y_d[d, :, hb], yo[:])

            with loop_ctx:
                # Software pipeline: chain z's P1/sigma interleaves with the
                # other chain's pending tail, anti-phasing the two chains on
                # the in-order engines.
                pend = {}
                for t in _repeat_steps(repeat):
                    for z in range(nh):
                        pt, tg = emit_P1(z, t)
                        zo = (z + 1) % nh
                        if zo in pend:
                            emit_P2(**pend.pop(zo))
                        sig = emit_sigma(z, pt)
                        pend[z] = dict(z=z, t=t, pt=pt, tg=tg, sig=sig)
                for z in list(pend):
                    emit_P2(**pend.pop(z))
    nc.finalize()
    return nc


def kernel(inputs, W_ih_enc, W_hh_enc, b_ih_enc, b_hh_enc,
           W_ih_dec, W_hh_dec, b_ih_dec, b_hh_dec, W_y, b_y,
           _trace=False, _perf_out=None):
    inputs = np.asarray(inputs, np.float32)
    f64 = np.float64
    encw = _pack_weights(np.asarray(W_hh_enc, f64), np.asarray(W_ih_enc, f64),
                         np.asarray(b_ih_enc, f64) + np.asarray(b_hh_enc, f64))
    Wihd = np.asarray(W_ih_dec, f64)
    Whhd = np.asarray(W_hh_dec, f64)
    bd = np.asarray(b_ih_dec, f64) + np.asarray(b_hh_dec, f64)
    Wyf = np.asarray(W_y, f64)
    byf = np.asarray(b_y, f64)
    dec0w = _pack_weights(Whhd, None, bd)
    decfw = _pack_weights(Whhd + Wihd @ Wyf, None, bd + Wihd @ byf)
    wyk = np.zeros((K, OUT), np.float32)
    wyk[0:H, :] = Wyf.T.astype(np.float32)
    wyk[K - 1, :] = byf.astype(np.float32)
    ones = np.ones((1, BS), np.float32)

    nc = _build_program()

    in_maps = []
    for core in range(NCORES):
        xs = inputs[:, core * BS:(core + 1) * BS, :]         # [T, BS, IN]
        xt = np.ascontiguousarray(xs.transpose(0, 2, 1))     # [T, IN, BS]
        in_maps.append({"x": xt, "encw": encw, "dec0w": dec0w,
                        "decfw": decfw, "wy": wyk, "ones": ones})

    import time as _time
    res = run_bass_kernel_spmd(nc, in_maps, core_ids=list(range(NCORES)),
                               trace=_trace)
    if _perf_out is not None:
        walls = []
        for _ in range(6):
            t0 = _time.time()
            res = run_bass_kernel_spmd(nc, in_maps,
                                       core_ids=list(range(NCORES)),
                                       trace=_trace)
            walls.append(time_ns := int((_time.time() - t0) * 1e9))
        _perf_out.update(exec_time_ns=res.exec_time_ns, walls_ns=walls,
                         trace=res.instructions_and_trace,
                         profile_json=res.profile_json)
    out = np.empty((DEC, B, OUT), np.float32)
    for core in range(NCORES):
        y = res.results[core]["y"]                           # [DEC, OUT, BS]
        out[:, core * BS:(core + 1) * BS, :] = y.transpose(0, 2, 1)
    return out



# revision 3
# speedup vs baseline: 1.0207x; 1.0207x over previous
"""Seq2seq LSTM (CoordinatePredictionModel) Trainium2 Bass kernel.

Model: 200-step LSTM encoder over [T=200, B=4096, IN=4], then 30-step
autoregressive LSTM decoder with output projection -> [30, B, OUT=4].

Sharding: pure data-parallel over batch. B=4096 -> 512 per core x 8 cores,
no collectives. Each core runs the full 230-step recurrence as `NH`
independent batch sub-chains, software-pipelined against each other so the
in-order engines stay busy.

Layout ("hidden on partitions"): per-step state lives transposed in one
SBUF tile s = [K=105, HB]: rows 0..99 = h^T, rows 100..103 = x^T (encoder
input; zero-weighted in decoder), row 104 = ones. Gate pre-activations for
chunk g come from one matmul
  psum[:, blk] = W_blk^T.T @ s          (K=105 contraction)
with W^T = [W_hh.T ; W_ih.T ; (b_ih+b_hh)] stacked on partitions, so the
input projection and both biases ride along in K. Gates are ordered
[g, i, f, o] and the g block is pre-scaled by 2 in the weights, so ONE
sigmoid ACTIVATE covers all four blocks: sigma(2g) encodes
tanh(g) = 2*sigma(2g) - 1, folded into the cell update on the DVE:
  c' = sigma(f)*c + 2*(sigma(2g) - 0.5)*sigma(i)

tanh(c') never touches the scalar engine: h = sigma(o) * tanh(c') is a
degree-3 odd minimax polynomial (|c| <= 1.6 by construction of this model;
max err 3e-3) evaluated by a custom DVE op:
  m = (p0 * sigma(o)) * c'                      [scalar_tensor_tensor]
  h = m * (((q3 u + q2) u + q1) u + 1), u=c'^2  [TANH_MUL_ANT, 8 ALU stages]

Decoder feedback y_prev = W_y h + b_y is folded into the recurrence:
  W_ih_dec @ y_prev + W_hh_dec @ h = (W_ih_dec W_y + W_hh_dec) @ h + W_ih_dec b_y
(valid from the second decoder step; the first uses y_prev = 0), so y is
only ever computed PSUM -> DRAM and never re-enters SBUF state.

Only DMA may write non-32-aligned partition bases; all engine writes here
start at partition 0 (x and ones rows are DMA-written).
"""

import os

import numpy as np

import concourse.bass as bass
import concourse.mybir as mybir
from concourse import bacc
from concourse import dve_ops
from concourse.dve_spec import Spec, Src0, Src1, C0, C1, C2, One, sq, lower
from concourse.dve_uop import DveOpSpec
from concourse.tile import TileContext
from concourse.bass_utils import run_bass_kernel_spmd

T, B, IN, OUT, H = 200, 4096, 4, 4, 100
DEC = 30
NCORES = 8
BS = B // NCORES          # 512 batch rows per core
K = H + IN + 1            # 105 = h + x + ones
F32 = mybir.dt.float32

# tanh(x) ~= x*(P0 + P1 u + P2 u^2 + P3 u^3), u = x^2, minimax on |x|<=1.6
_TP = (0.99622347, -0.30820215, 0.08443338, -0.01110886)
_TQ = (_TP[1] / _TP[0], _TP[2] / _TP[0], _TP[3] / _TP[0])


def _cfg(name, default):
    v = os.environ.get(name)
    return default if v is None else type(default)(v)


NH = _cfg("K_NH", 2)                  # independent batch chains per core
SPLIT_SIGMA = _cfg("K_SPLIT", 0)      # 1: two ACTs [g,i] + [f,o]
W_ENG = _cfg("K_WENG", "gpsimd")      # engine for w = sigma(f)*c
V_ENG = _cfg("K_VENG", "vector")
M_ENG = _cfg("K_MENG", "vector")
CUSTOM_H = _cfg("K_CUSTOM_H", 1)      # 0: tanh(c) via ACT + plain mult
C_BF16 = _cfg("K_CBF16", 0)           # cell state dtype bf16 (else fp32)
MM_DT = _cfg("K_MMDT", "bf16")        # matmul operand dtype: bf16|f32r
HIPRI_H = _cfg("K_HIPRI", 1)

BF16 = mybir.dt.bfloat16
MDT = BF16 if MM_DT == "bf16" else mybir.dt.float32r
EWDT = BF16                           # sigma outputs / v / m / h
CDT = BF16 if C_BF16 else F32

# gate order [g, i, f, o]; g block pre-scaled by 2 (tanh via sigmoid)
_PERM = np.concatenate([np.arange(200, 300), np.arange(0, 100),
                        np.arange(100, 200), np.arange(300, 400)])
_GSCALE = np.concatenate([np.full(100, 2.0), np.ones(300)])


def _np_dt(dt):
    return mybir.dt.np(dt)


def _pack_weights(W_hh, W_ih, bias):
    """[K=105, 4H] stacked lhsT, gate order [g,i,f,o], g block x2."""
    Wk = np.zeros((K, 4 * H), np.float64)
    Wk[0:H, :] = W_hh.T[:, _PERM]
    if W_ih is not None:
        Wk[H:H + W_ih.shape[1], :] = W_ih.T[:, _PERM]
    Wk[K - 1, :] = bias[_PERM]
    Wk *= _GSCALE[None, :]
    return Wk.astype(_np_dt(MDT))


def _register_tanh_mul():
    """out = in1 * (((u*q3 + q2)*u + q1)*u + 1), u = in0^2."""
    name = "TANH_MUL_ANT"
    for o in dve_ops.OPS:
        if o.name == name:
            return o
    u = sq(Src0)
    spec = Spec(
        body=(((u * C0 + C1) * u + C2) * u + One) * Src1,
        reference=lambda in0, in1, s0, s1, imm2: (
            (((in0 * in0) * s0 + s1) * (in0 * in0) + imm2) * (in0 * in0) + 1.0
        ) * in1,
    )
    opcode = dve_ops._CUSTOM_DVE_ROW_BASE + len(dve_ops.OPS)
    shas = {
        ver: DveOpSpec(name=name, opcode=opcode, uops=lower(spec, ver=ver),
                       rd1_en=True).sha(ver)
        for ver in ("v3", "v4")
    }
    op = dve_ops.DveOp(name, spec, subdim=False, uops_sha=shas)
    dve_ops.OPS.append(op)
    dve_ops._SUB_OPCODE_FOR_NAME[name] = opcode
    return op


_TANH_MUL = _register_tanh_mul()


def _build_program(nh=NH):
    assert BS % nh == 0
    HB = BS // nh
    nc = bacc.Bacc("TRN2", debug=False, num_devices=NCORES)

    x_d = nc.dram_tensor("x", (T, IN, BS), MDT, kind="ExternalInput").ap()
    encw_d = nc.dram_tensor("encw", (K, 4 * H), MDT, kind="ExternalInput").ap()
    dec0w_d = nc.dram_tensor("dec0w", (K, 4 * H), MDT, kind="ExternalInput").ap()
    decfw_d = nc.dram_tensor("decfw", (K, 4 * H), MDT, kind="ExternalInput").ap()
    wy_d = nc.dram_tensor("wy", (K, OUT), MDT, kind="ExternalInput").ap()
    ones_d = nc.dram_tensor("ones", (1, BS), MDT, kind="ExternalInput").ap()
    y_d = nc.dram_tensor("y", (DEC, OUT, BS), F32, kind="ExternalOutput").ap()

    AF = mybir.ActivationFunctionType
    ALU = mybir.AluOpType
    veng = {"vector": None, "gpsimd": None}

    with TileContext(nc) as tc:
        veng = {"vector": nc.vector, "gpsimd": nc.gpsimd}
        w_eng, v_eng, m_eng = veng[W_ENG], veng[V_ENG], veng[M_ENG]
        with (
            tc.tile_pool(name="const", bufs=1) as constp,
            tc.tile_pool(name="state", bufs=1) as statep,
            tc.tile_pool(name="work", bufs=3) as work,
            tc.tile_pool(name="psum", bufs=1, space="PSUM") as psump,
        ):
            encw = constp.tile([K, 4 * H], MDT, tag="encw")
            dec0w = constp.tile([K, 4 * H], MDT, tag="dec0w")
            decfw = constp.tile([K, 4 * H], MDT, tag="decfw")
            wy = constp.tile([K, OUT], MDT, tag="wy")
            nc.sync.dma_start(encw[:], encw_d[:])
            nc.sync.dma_start(dec0w[:], dec0w_d[:])
            nc.sync.dma_start(decfw[:], decfw_d[:])
            nc.sync.dma_start(wy[:], wy_d[:])

            sz = [[statep.tile([K, HB], MDT, tag=f"s{p}{z}", name=f"s{p}{z}")
                   for p in range(2)] for z in range(nh)]
            cz = [statep.tile([H, HB], CDT, tag=f"c{z}", name=f"c{z}")
                  for z in range(nh)]
            ptz = [psump.tile([H, 4 * HB], F32, tag=f"pt{z}", name=f"pt{z}")
                   for z in range(nh)]

            for z in range(nh):
                hb = slice(z * HB, (z + 1) * HB)
                for p in range(2):
                    nc.gpsimd.memset(
                        sz[z][p][0:H, :].bitcast(mybir.dt.uint16 if MDT == BF16
                                                 else mybir.dt.uint32), 0)
                    nc.sync.dma_start(sz[z][p][K - 1:K, :], ones_d[0:1, hb])
                    nc.sync.dma_start(sz[z][p][H:H + IN, :], x_d[p, :, hb])
                nc.vector.memset(cz[z][:], 0.0)

            def emit_P1(z, t):
                """Gate matmuls (order g,i,f,o) + x prefetch for t+2."""
                W = encw if t < T else (dec0w if t == T else decfw)
                prev = sz[z][t % 2]
                pt = ptz[z]
                for gi in range(4):
                    nc.tensor.matmul(pt[:, gi * HB:(gi + 1) * HB],
                                     W[:, gi * H:(gi + 1) * H],
                                     prev[:], start=True, stop=True)
                if t + 2 < T:
                    hb = slice(z * HB, (z + 1) * HB)
                    nc.sync.dma_start(sz[z][t % 2][H:H + IN, :],
                                      x_d[t + 2, :, hb])
                return pt

            def emit_sigma(z, pt):
                S = work.tile([H, 4 * HB], EWDT, tag=f"S{z}", name=f"S{z}")
                if SPLIT_SIGMA:
                    nc.scalar.activation(S[:, 0:2 * HB], pt[:, 0:2 * HB],
                                         AF.Sigmoid)
                    nc.scalar.activation(S[:, 2 * HB:4 * HB],
                                         pt[:, 2 * HB:4 * HB], AF.Sigmoid)
                else:
                    nc.scalar.activation(S[:], pt[:], AF.Sigmoid)
                return S

            def emit_P2(z, t, pt, S):
                """Cell/hidden tail for chain z, step t.

                S blocks: [2g | i | f | o]."""
                c = cz[z]
                Sg, Si = S[:, 0:HB], S[:, HB:2 * HB]
                Sf, So = S[:, 2 * HB:3 * HB], S[:, 3 * HB:4 * HB]
                v = work.tile([H, HB], EWDT, tag=f"v{z}", name=f"v{z}")
                v_eng.scalar_tensor_tensor(v[:], Sg, -0.5, Si,
                                           ALU.add, ALU.mult)
                w = work.tile([H, HB], CDT, tag=f"w{z}", name=f"w{z}")
                w_eng.tensor_mul(w[:], Sf, c[:])
                nc.vector.scalar_tensor_tensor(c[:], v[:], 2.0, w[:],
                                               ALU.mult, ALU.add)
                nxt = sz[z][(t + 1) % 2]
                import contextlib
                hp = tc.high_priority() if HIPRI_H else contextlib.nullcontext()
                with hp:
                    if CUSTOM_H:
                        m = work.tile([H, HB], EWDT, tag=f"m{z}", name=f"m{z}")
                        m_eng.scalar_tensor_tensor(m[:], So, float(_TP[0]),
                                                   c[:], ALU.mult, ALU.mult)
                        nc.vector._custom_dve(
                            _TANH_MUL, out=nxt[0:H, :], in0=c[:], in1=m[:],
                            s0=float(_TQ[2]), s1=float(_TQ[1]),
                            imm2=float(_TQ[0]))
                    else:
                        tct = work.tile([H, HB], EWDT, tag=f"tct{z}",
                                        name=f"tct{z}")
                        nc.scalar.activation(tct[:], c[:], AF.Tanh)
                        nc.vector.tensor_mul(nxt[0:H, :], So, tct[:])
                hb = slice(z * HB, (z + 1) * HB)
                if t == T - 1:
                    nc.vector.memset(c[:], 0.0)
                elif t >= T:
                    d = t - T
                    yp = pt[0:OUT, 0:HB]
                    nc.tensor.matmul(yp, wy[:], nxt[:], start=True, stop=True)
                    yo = work.tile([OUT, HB], F32, tag=f"yo{z}", name=f"yo{z}")
                    nc.vector.tensor_copy(yo[:], yp)
                    nc.sync.dma_start(y_d[d, :, hb], yo[:])

            # Software pipeline: chain z's P1/sigma interleaves with the
            # previous chain's pending tail, anti-phasing the chains on
            # the in-order engines.
            pend = {}
            for t in range(T + DEC):
                for z in range(nh):
                    pt = emit_P1(z, t)
                    zo = (z + 1) % nh
                    if zo in pend:
                        emit_P2(**pend.pop(zo))
                    S = emit_sigma(z, pt)
                    pend[z] = dict(z=z, t=t, pt=pt, S=S)
            for z in list(pend):
                emit_P2(**pend.pop(z))
    nc.finalize()
    return nc


def kernel(inputs, W_ih_enc, W_hh_enc, b_ih_enc, b_hh_enc,
           W_ih_dec, W_hh_dec, b_ih_dec, b_hh_dec, W_y, b_y,
           _trace=False, _perf_out=None):
    f64 = np.float64
    encw = _pack_weights(np.asarray(W_hh_enc, f64), np.asarray(W_ih_enc, f64),
                         np.asarray(b_ih_enc, f64) + np.asarray(b_hh_enc, f64))
    Wihd = np.asarray(W_ih_dec, f64)
    Whhd = np.asarray(W_hh_dec, f64)
    bd = np.asarray(b_ih_dec, f64) + np.asarray(b_hh_dec, f64)
    Wyf = np.asarray(W_y, f64)
    byf = np.asarray(b_y, f64)
    dec0w = _pack_weights(Whhd, None, bd)
    decfw = _pack_weights(Whhd + Wihd @ Wyf, None, bd + Wihd @ byf)
    ndt = _np_dt(MDT)
    wyk = np.zeros((K, OUT), np.float64)
    wyk[0:H, :] = Wyf.T
    wyk[K - 1, :] = byf
    wyk = wyk.astype(ndt)
    ones = np.ones((1, BS), ndt)

    nc = _build_program()

    inputs = np.asarray(inputs, np.float32)
    in_maps = []
    for core in range(NCORES):
        xs = inputs[:, core * BS:(core + 1) * BS, :]         # [T, BS, IN]
        xt = np.ascontiguousarray(xs.transpose(0, 2, 1)).astype(ndt)
        in_maps.append({"x": xt, "encw": encw, "dec0w": dec0w,
                        "decfw": decfw, "wy": wyk, "ones": ones})

    import time as _time
    res = run_bass_kernel_spmd(nc, in_maps, core_ids=list(range(NCORES)),
                               trace=_trace)
    if _perf_out is not None:
        walls = []
        for _ in range(6):
            t0 = _time.time()
            res = run_bass_kernel_spmd(nc, in_maps,
                                       core_ids=list(range(NCORES)),
                                       trace=_trace)
            walls.append(int((_time.time() - t0) * 1e9))
        _perf_out.update(exec_time_ns=res.exec_time_ns, walls_ns=walls,
                         trace=res.instructions_and_trace,
                         profile_json=res.profile_json)
    out = np.empty((DEC, B, OUT), np.float32)
    for core in range(NCORES):
        y = res.results[core]["y"]                           # [DEC, OUT, BS]
        out[:, core * BS:(core + 1) * BS, :] = y.transpose(0, 2, 1)
    return out


# revision 4
# speedup vs baseline: 1.1650x; 1.1413x over previous
"""Seq2seq LSTM (CoordinatePredictionModel) Trainium2 Bass kernel.

Model: 200-step LSTM encoder over [T=200, B=4096, IN=4], then 30-step
autoregressive LSTM decoder with output projection -> [30, B, OUT=4].

Sharding: pure data-parallel over batch. B=4096 -> 512 per core x 8 cores,
no collectives. Each core runs the full 230-step recurrence as `NH`
independent batch sub-chains, software-pipelined against each other so the
in-order engines stay busy.

Layout ("hidden on partitions"): per-step state lives transposed in one
SBUF tile s = [K=105, HB]: rows 0..99 = h^T, rows 100..103 = x^T (encoder
input; zero-weighted in decoder), row 104 = ones. Gate pre-activations for
block g come from one matmul
  psum[:, blk] = W_blk^T.T @ s          (K=105 contraction)
with W^T = [W_hh.T ; W_ih.T ; (b_ih+b_hh)] stacked on partitions, so the
input projection and both biases ride along in K.

Activation-engine work is ONE sigmoid ACTIVATE per chain-step (two when
SPLIT_SIGMA pipelines it against the tail): gates are ordered [f, o, g, i]
and the g block is pre-scaled by 2 in the weights, so sigma(2g) encodes
tanh(g) = 2*sigma(2g) - 1.

The cell state is tracked at HALF SCALE, c* = c/2, which makes the update
a plain add (the usual 2x folds away):
  v  = (sigma(2g) - 0.5) * sigma(i)     [scalar_tensor_tensor]
  w  = sigma(f) * c*                     [tensor_tensor]
  c*' = v + w                            [tensor_tensor]

tanh never touches the scalar engine: h = sigma(o) * tanh(2 c*') is a
degree-3 odd minimax polynomial on |c*| <= 0.75 (max err 2.1e-3), factored
(quadratic)x(linear) so it fits one 8-stage custom DVE op with no
pre-multiply:
  h/k = sigma(o) * c* * ((u + a) u + b) * (u + c),   u = c*^2
The scale k is folded into every h-consuming weight row (W_hh, W_y), so
the stored hidden state is h* = h/k.

Decoder feedback y_prev = W_y h + b_y is folded into the recurrence:
  W_ih_dec @ y_prev + W_hh_dec @ h = (W_ih_dec W_y + W_hh_dec) @ h + W_ih_dec b_y
(valid from the second decoder step; the first uses y_prev = 0), so y is
only ever computed PSUM -> DRAM and never re-enters SBUF state.

Only DMA may write non-32-aligned partition bases; all engine writes here
start at partition 0 (x and ones rows are DMA-written).
"""

import os

import numpy as np

import concourse.bass as bass
import concourse.mybir as mybir
from concourse import bacc
from concourse import dve_ops
from concourse.dve_spec import Spec, Src0, Src1, C0, C1, C2, sq, lower
from concourse.dve_uop import DveOpSpec
from concourse.tile import TileContext
from concourse.bass_utils import run_bass_kernel_spmd

T, B, IN, OUT, H = 200, 4096, 4, 4, 100
DEC = 30
NCORES = 8
BS = B // NCORES          # 512 batch rows per core
K = H + IN + 1            # 105 = h + x + ones
F32 = mybir.dt.float32

# tanh(2x) ~= x * TK * ((u + TA) u + TB) * (u + TC), u = x^2, |x| <= 0.75
TK = -1.6587196319980422
TA = -0.45058800513926967
TB = 0.9292510848600172
TC = -1.2941383843055507


def _cfg(name, default):
    v = os.environ.get(name)
    return default if v is None else type(default)(v)


NH = _cfg("K_NH", 2)                  # independent batch chains per core
SPLIT_SIGMA = _cfg("K_SPLIT", 1)      # 1: two ACTs [f,o] + [2g,i]
W_ENG = _cfg("K_WENG", "vector")      # engine for w = sigma(f)*c
V_ENG = _cfg("K_VENG", "vector")
C_BF16 = _cfg("K_CBF16", 1)           # cell state dtype bf16 (else fp32)
MM_DT = _cfg("K_MMDT", "bf16")        # matmul operand dtype: bf16|f32r
HIPRI_H = _cfg("K_HIPRI", 0)

BF16 = mybir.dt.bfloat16
MDT = BF16 if MM_DT == "bf16" else mybir.dt.float32r
EWDT = BF16                           # sigma outputs / v / h
CDT = BF16 if C_BF16 else F32

# gate order [f, o, g, i]; g block pre-scaled by 2 (tanh via sigmoid)
_PERM = np.concatenate([np.arange(100, 200), np.arange(300, 400),
                        np.arange(200, 300), np.arange(0, 100)])
_GSCALE = np.concatenate([np.ones(200), np.full(100, 2.0), np.ones(100)])
BF, BO, BG, BI = 0, 1, 2, 3           # block indices after _PERM


def _np_dt(dt):
    return mybir.dt.np(dt)


def _pack_weights(W_hh, W_ih, bias):
    """[K=105, 4H] stacked lhsT, gate order [f,o,g,i], g block x2.

    Rows 0:H act on the stored hidden state h* = h/TK, so they carry an
    extra factor TK."""
    Wk = np.zeros((K, 4 * H), np.float64)
    Wk[0:H, :] = TK * W_hh.T[:, _PERM]
    if W_ih is not None:
        Wk[H:H + W_ih.shape[1], :] = W_ih.T[:, _PERM]
    Wk[K - 1, :] = bias[_PERM]
    Wk *= _GSCALE[None, :]
    return Wk.astype(_np_dt(MDT))


def _register_tanh2_mul():
    """out = in0 * in1 * ((u + s0) u + s1) * (u + imm2), u = in0^2."""
    name = "TANH2_MUL_ANT"
    for o in dve_ops.OPS:
        if o.name == name:
            return o
    u = sq(Src0)
    spec = Spec(
        body=((u + C0) * u + C1) * (u + C2) * Src0 * Src1,
        reference=lambda in0, in1, s0, s1, imm2: (
            ((in0 * in0 + s0) * (in0 * in0) + s1)
            * (in0 * in0 + imm2) * in0 * in1
        ),
    )
    opcode = dve_ops._CUSTOM_DVE_ROW_BASE + len(dve_ops.OPS)
    shas = {
        ver: DveOpSpec(name=name, opcode=opcode, uops=lower(spec, ver=ver),
                       rd1_en=True).sha(ver)
        for ver in ("v3", "v4")
    }
    op = dve_ops.DveOp(name, spec, subdim=False, uops_sha=shas)
    dve_ops.OPS.append(op)
    dve_ops._SUB_OPCODE_FOR_NAME[name] = opcode
    return op


_TANH2_MUL = _register_tanh2_mul()


def _build_program(nh=NH):
    assert BS % nh == 0
    HB = BS // nh
    nc = bacc.Bacc("TRN2", debug=False, num_devices=NCORES)

    x_d = nc.dram_tensor("x", (T, IN, BS), MDT, kind="ExternalInput").ap()
    encw_d = nc.dram_tensor("encw", (K, 4 * H), MDT, kind="ExternalInput").ap()
    dec0w_d = nc.dram_tensor("dec0w", (K, 4 * H), MDT, kind="ExternalInput").ap()
    decfw_d = nc.dram_tensor("decfw", (K, 4 * H), MDT, kind="ExternalInput").ap()
    wy_d = nc.dram_tensor("wy", (K, OUT), MDT, kind="ExternalInput").ap()
    ones_d = nc.dram_tensor("ones", (1, BS), MDT, kind="ExternalInput").ap()
    y_d = nc.dram_tensor("y", (DEC, OUT, BS), F32, kind="ExternalOutput").ap()

    AF = mybir.ActivationFunctionType
    ALU = mybir.AluOpType

    with TileContext(nc) as tc:
        veng = {"vector": nc.vector, "gpsimd": nc.gpsimd}
        w_eng, v_eng = veng[W_ENG], veng[V_ENG]
        with (
            tc.tile_pool(name="const", bufs=1) as constp,
            tc.tile_pool(name="state", bufs=1) as statep,
            tc.tile_pool(name="work", bufs=3) as work,
            tc.tile_pool(name="psum", bufs=1, space="PSUM") as psump,
        ):
            encw = constp.tile([K, 4 * H], MDT, tag="encw")
            dec0w = constp.tile([K, 4 * H], MDT, tag="dec0w")
            decfw = constp.tile([K, 4 * H], MDT, tag="decfw")
            wy = constp.tile([K, OUT], MDT, tag="wy")
            nc.sync.dma_start(encw[:], encw_d[:])
            nc.sync.dma_start(dec0w[:], dec0w_d[:])
            nc.sync.dma_start(decfw[:], decfw_d[:])
            nc.sync.dma_start(wy[:], wy_d[:])

            sz = [[statep.tile([K, HB], MDT, tag=f"s{p}{z}", name=f"s{p}{z}")
                   for p in range(2)] for z in range(nh)]
            cz = [statep.tile([H, HB], CDT, tag=f"c{z}", name=f"c{z}")
                  for z in range(nh)]
            ptz = [psump.tile([H, 4 * HB], F32, tag=f"pt{z}", name=f"pt{z}")
                   for z in range(nh)]

            for z in range(nh):
                hb = slice(z * HB, (z + 1) * HB)
                for p in range(2):
                    nc.gpsimd.memset(
                        sz[z][p][0:H, :].bitcast(mybir.dt.uint16 if MDT == BF16
                                                 else mybir.dt.uint32), 0)
                    nc.sync.dma_start(sz[z][p][K - 1:K, :], ones_d[0:1, hb])
                    nc.sync.dma_start(sz[z][p][H:H + IN, :], x_d[p, :, hb])
                nc.vector.memset(cz[z][:], 0.0)

            def emit_P1(z, t):
                """Gate matmuls (order f,o,g,i) + x prefetch for t+2."""
                W = encw if t < T else (dec0w if t == T else decfw)
                prev = sz[z][t % 2]
                pt = ptz[z]
                for gi in range(4):
                    nc.tensor.matmul(pt[:, gi * HB:(gi + 1) * HB],
                                     W[:, gi * H:(gi + 1) * H],
                                     prev[:], start=True, stop=True)
                if t + 2 < T:
                    hb = slice(z * HB, (z + 1) * HB)
                    nc.sync.dma_start(sz[z][t % 2][H:H + IN, :],
                                      x_d[t + 2, :, hb])
                return pt

            def emit_sigma(z, pt):
                S = work.tile([H, 4 * HB], EWDT, tag=f"S{z}", name=f"S{z}")
                if SPLIT_SIGMA:
                    nc.scalar.activation(S[:, 0:2 * HB], pt[:, 0:2 * HB],
                                         AF.Sigmoid)
                    nc.scalar.activation(S[:, 2 * HB:4 * HB],
                                         pt[:, 2 * HB:4 * HB], AF.Sigmoid)
                else:
                    nc.scalar.activation(S[:], pt[:], AF.Sigmoid)
                return S

            def emit_P2(z, t, pt, S):
                """Cell/hidden tail for chain z, step t.

                S blocks: [f | o | 2g | i]."""
                c = cz[z]
                Sf, So = S[:, 0:HB], S[:, HB:2 * HB]
                Sg, Si = S[:, 2 * HB:3 * HB], S[:, 3 * HB:4 * HB]
                w = work.tile([H, HB], CDT, tag=f"w{z}", name=f"w{z}")
                w_eng.tensor_mul(w[:], Sf, c[:])
                v = work.tile([H, HB], EWDT, tag=f"v{z}", name=f"v{z}")
                v_eng.scalar_tensor_tensor(v[:], Sg, -0.5, Si,
                                           ALU.add, ALU.mult)
                nc.vector.tensor_add(c[:], v[:], w[:])
                nxt = sz[z][(t + 1) % 2]
                import contextlib
                hp = tc.high_priority() if HIPRI_H else contextlib.nullcontext()
                with hp:
                    nc.vector._custom_dve(
                        _TANH2_MUL, out=nxt[0:H, :], in0=c[:], in1=So,
                        s0=float(TA), s1=float(TB), imm2=float(TC))
                hb = slice(z * HB, (z + 1) * HB)
                if t == T - 1:
                    nc.vector.memset(c[:], 0.0)
                elif t >= T:
                    d = t - T
                    yp = pt[0:OUT, 0:HB]
                    nc.tensor.matmul(yp, wy[:], nxt[:], start=True, stop=True)
                    yo = work.tile([OUT, HB], F32, tag=f"yo{z}", name=f"yo{z}")
                    nc.vector.tensor_copy(yo[:], yp)
                    nc.sync.dma_start(y_d[d, :, hb], yo[:])

            # Software pipeline: chain z's P1/sigma interleaves with the
            # previous chain's pending tail, anti-phasing the chains on
            # the in-order engines.
            pend = {}
            for t in range(T + DEC):
                for z in range(nh):
                    pt = emit_P1(z, t)
                    zo = (z + 1) % nh
                    if zo in pend:
                        emit_P2(**pend.pop(zo))
                    S = emit_sigma(z, pt)
                    pend[z] = dict(z=z, t=t, pt=pt, S=S)
            for z in list(pend):
                emit_P2(**pend.pop(z))
    nc.finalize()
    return nc


def kernel(inputs, W_ih_enc, W_hh_enc, b_ih_enc, b_hh_enc,
           W_ih_dec, W_hh_dec, b_ih_dec, b_hh_dec, W_y, b_y,
           _trace=False, _perf_out=None):
    f64 = np.float64
    encw = _pack_weights(np.asarray(W_hh_enc, f64), np.asarray(W_ih_enc, f64),
                         np.asarray(b_ih_enc, f64) + np.asarray(b_hh_enc, f64))
    Wihd = np.asarray(W_ih_dec, f64)
    Whhd = np.asarray(W_hh_dec, f64)
    bd = np.asarray(b_ih_dec, f64) + np.asarray(b_hh_dec, f64)
    Wyf = np.asarray(W_y, f64)
    byf = np.asarray(b_y, f64)
    dec0w = _pack_weights(Whhd, None, bd)
    decfw = _pack_weights(Whhd + Wihd @ Wyf, None, bd + Wihd @ byf)
    ndt = _np_dt(MDT)
    wyk = np.zeros((K, OUT), np.float64)
    wyk[0:H, :] = TK * Wyf.T
    wyk[K - 1, :] = byf
    wyk = wyk.astype(ndt)
    ones = np.ones((1, BS), ndt)

    nc = _build_program()

    inputs = np.asarray(inputs, np.float32)
    in_maps = []
    for core in range(NCORES):
        xs = inputs[:, core * BS:(core + 1) * BS, :]         # [T, BS, IN]
        xt = np.ascontiguousarray(xs.transpose(0, 2, 1)).astype(ndt)
        in_maps.append({"x": xt, "encw": encw, "dec0w": dec0w,
                        "decfw": decfw, "wy": wyk, "ones": ones})

    import time as _time
    res = run_bass_kernel_spmd(nc, in_maps, core_ids=list(range(NCORES)),
                               trace=_trace)
    if _perf_out is not None:
        walls = []
        for _ in range(6):
            t0 = _time.time()
            res = run_bass_kernel_spmd(nc, in_maps,
                                       core_ids=list(range(NCORES)),
                                       trace=_trace)
            walls.append(int((_time.time() - t0) * 1e9))
        _perf_out.update(exec_time_ns=res.exec_time_ns, walls_ns=walls,
                         trace=res.instructions_and_trace,
                         profile_json=res.profile_json)
    out = np.empty((DEC, B, OUT), np.float32)
    for core in range(NCORES):
        y = res.results[core]["y"]                           # [DEC, OUT, BS]
        out[:, core * BS:(core + 1) * BS, :] = y.transpose(0, 2, 1)
    return out


# revision 7
# speedup vs baseline: 1.4596x; 1.2529x over previous
"""Seq2seq LSTM (CoordinatePredictionModel) Trainium2 Bass kernel.

Model: 200-step LSTM encoder over [T=200, B=4096, IN=4], then 30-step
autoregressive LSTM decoder with output projection -> [30, B, OUT=4].

Sharding: pure data-parallel over batch. B=4096 -> 512 per core x 8 cores,
no collectives. Each core runs the full 230-step recurrence as `NH`
independent batch sub-chains, software-pipelined against each other so the
in-order engines stay busy.

Layout ("hidden on partitions"): per-step state lives transposed in one
SBUF tile s = [K=105, HB]: rows 0..99 = h^T, rows 100..103 = x^T (encoder
input; zero-weighted in decoder), row 104 = ones. Gate pre-activations for
block g come from one matmul
  psum[:, blk] = W_blk^T.T @ s          (K=105 contraction)
with W^T = [W_hh.T ; W_ih.T ; (b_ih+b_hh)] stacked on partitions, so the
input projection and both biases ride along in K.

Activation-engine work is ONE sigmoid ACTIVATE per chain-step (two when
SPLIT_SIGMA pipelines it against the tail): gates are ordered [f, o, g, i]
and the g block is pre-scaled by 2 in the weights, so sigma(2g) encodes
tanh(g) = 2*sigma(2g) - 1.

The cell state is tracked at HALF SCALE, c* = c/2, which makes the update
a plain add (the usual 2x folds away):
  v  = (sigma(2g) - 0.5) * sigma(i)     [scalar_tensor_tensor]
  w  = sigma(f) * c*                     [tensor_tensor]
  c*' = v + w                            [tensor_tensor]

tanh never touches the scalar engine: h = sigma(o) * tanh(2 c*') is a
degree-3 odd minimax polynomial on |c*| <= 0.75 (max err 2.1e-3), factored
(quadratic)x(linear) so it fits one 8-stage custom DVE op with no
pre-multiply:
  h/k = sigma(o) * c* * ((u + a) u + b) * (u + c),   u = c*^2
The scale k is folded into every h-consuming weight row (W_hh, W_y), so
the stored hidden state is h* = h/k.

Decoder feedback y_prev = W_y h + b_y is folded into the recurrence:
  W_ih_dec @ y_prev + W_hh_dec @ h = (W_ih_dec W_y + W_hh_dec) @ h + W_ih_dec b_y
(valid from the second decoder step; the first uses y_prev = 0), so y is
only ever computed PSUM -> DRAM and never re-enters SBUF state.

Only DMA may write non-32-aligned partition bases; all engine writes here
start at partition 0 (x and ones rows are DMA-written).
"""

import os

import numpy as np

import concourse.bass as bass
import concourse.mybir as mybir
from concourse import bacc
from concourse import dve_ops
from concourse.dve_spec import Spec, Src0, Src1, C0, C1, C2, sq, lower
from concourse.dve_uop import DveOpSpec
from concourse.tile import TileContext
from concourse.bass_utils import run_bass_kernel_spmd

T, B, IN, OUT, H = 200, 4096, 4, 4, 100
DEC = 30
NCORES = 8
BS = B // NCORES          # 512 batch rows per core
K = H + IN + 1            # 105 = h + x + ones
F32 = mybir.dt.float32

# tanh(2x) ~= x * TK * ((u + TA) u + TB) * (u + TC), u = x^2, |x| <= 0.75
TK = -1.6587196319980422
TA = -0.45058800513926967
TB = 0.9292510848600172
TC = -1.2941383843055507


def _cfg(name, default):
    v = os.environ.get(name)
    return default if v is None else type(default)(v)


NH = _cfg("K_NH", 2)                  # independent batch chains per core
SPLIT_SIGMA = _cfg("K_SPLIT", 0)      # 1: two ACTs [f,o] + [2g,i]
WARM = _cfg("K_WARM", 0)              # junk MMs per burst to keep PE at 2.4GHz
W_ENG = _cfg("K_WENG", "vector")      # engine for w = sigma(f)*c
V_ENG = _cfg("K_VENG", "vector")
C_BF16 = _cfg("K_CBF16", 1)           # cell state dtype bf16 (else fp32)
MM_DT = _cfg("K_MMDT", "bf16")        # matmul operand dtype: bf16|f32r
HIPRI_H = _cfg("K_HIPRI", 0)

BF16 = mybir.dt.bfloat16
MDT = BF16 if MM_DT == "bf16" else mybir.dt.float32r
EWDT = BF16                           # sigma outputs / v / h
CDT = BF16 if C_BF16 else F32

# gate order [f, o, g, i]; g block pre-scaled by 2 (tanh via sigmoid)
_PERM = np.concatenate([np.arange(100, 200), np.arange(300, 400),
                        np.arange(200, 300), np.arange(0, 100)])
_GSCALE = np.concatenate([np.ones(200), np.full(100, 2.0), np.ones(100)])
BF, BO, BG, BI = 0, 1, 2, 3           # block indices after _PERM


def _np_dt(dt):
    return mybir.dt.np(dt)


def _pack_weights(W_hh, W_ih, bias):
    """[K=105, 4H] stacked lhsT, gate order [f,o,g,i], g block x2.

    Rows 0:H act on the stored hidden state h* = h/TK, so they carry an
    extra factor TK."""
    Wk = np.zeros((K, 4 * H), np.float64)
    Wk[0:H, :] = TK * W_hh.T[:, _PERM]
    if W_ih is not None:
        Wk[H:H + W_ih.shape[1], :] = W_ih.T[:, _PERM]
    Wk[K - 1, :] = bias[_PERM]
    Wk *= _GSCALE[None, :]
    return Wk.astype(_np_dt(MDT))


def _register_tanh2_mul():
    """out = in0 * in1 * ((u + s0) u + s1) * (u + imm2), u = in0^2."""
    name = "TANH2_MUL_ANT"
    for o in dve_ops.OPS:
        if o.name == name:
            return o
    u = sq(Src0)
    spec = Spec(
        body=((u + C0) * u + C1) * (u + C2) * Src0 * Src1,
        reference=lambda in0, in1, s0, s1, imm2: (
            ((in0 * in0 + s0) * (in0 * in0) + s1)
            * (in0 * in0 + imm2) * in0 * in1
        ),
    )
    opcode = dve_ops._CUSTOM_DVE_ROW_BASE + len(dve_ops.OPS)
    shas = {
        ver: DveOpSpec(name=name, opcode=opcode, uops=lower(spec, ver=ver),
                       rd1_en=True).sha(ver)
        for ver in ("v3", "v4")
    }
    op = dve_ops.DveOp(name, spec, subdim=False, uops_sha=shas)
    dve_ops.OPS.append(op)
    dve_ops._SUB_OPCODE_FOR_NAME[name] = opcode
    return op


_TANH2_MUL = _register_tanh2_mul()


def _build_program(nh=NH):
    assert BS % nh == 0
    HB = BS // nh
    nc = bacc.Bacc("TRN2", debug=False, num_devices=NCORES)

    x_d = nc.dram_tensor("x", (T, IN, BS), MDT, kind="ExternalInput").ap()
    encw_d = nc.dram_tensor("encw", (K, 4 * H), MDT, kind="ExternalInput").ap()
    dec0w_d = nc.dram_tensor("dec0w", (K, 4 * H), MDT, kind="ExternalInput").ap()
    decfw_d = nc.dram_tensor("decfw", (K, 4 * H), MDT, kind="ExternalInput").ap()
    wy_d = nc.dram_tensor("wy", (K, OUT), MDT, kind="ExternalInput").ap()
    ones_d = nc.dram_tensor("ones", (1, BS), MDT, kind="ExternalInput").ap()
    y_d = nc.dram_tensor("y", (DEC, OUT, BS), F32, kind="ExternalOutput").ap()

    AF = mybir.ActivationFunctionType
    ALU = mybir.AluOpType

    with TileContext(nc) as tc:
        veng = {"vector": nc.vector, "gpsimd": nc.gpsimd}
        w_eng, v_eng = veng[W_ENG], veng[V_ENG]
        with (
            tc.tile_pool(name="const", bufs=1) as constp,
            tc.tile_pool(name="state", bufs=1) as statep,
            tc.tile_pool(name="work", bufs=3) as work,
            tc.tile_pool(name="psum", bufs=1, space="PSUM") as psump,
        ):
            encw = constp.tile([K, 4 * H], MDT, tag="encw")
            dec0w = constp.tile([K, 4 * H], MDT, tag="dec0w")
            decfw = constp.tile([K, 4 * H], MDT, tag="decfw")
            wy = constp.tile([K, OUT], MDT, tag="wy")
            nc.sync.dma_start(encw[:], encw_d[:])
            nc.sync.dma_start(dec0w[:], dec0w_d[:])
            nc.sync.dma_start(decfw[:], decfw_d[:])
            nc.sync.dma_start(wy[:], wy_d[:])

            sz = [[statep.tile([K, HB], MDT, tag=f"s{p}{z}", name=f"s{p}{z}")
                   for p in range(2)] for z in range(nh)]
            cz = [statep.tile([H, HB], CDT, tag=f"c{z}", name=f"c{z}")
                  for z in range(nh)]
            ptz = [psump.tile([H, 4 * HB], F32, tag=f"pt{z}", name=f"pt{z}")
                   for z in range(nh)]
            junkp = (psump.tile([H, 4 * H], F32, tag="junk", name="junk")
                     if WARM else None)

            for z in range(nh):
                hb = slice(z * HB, (z + 1) * HB)
                for p in range(2):
                    nc.gpsimd.memset(
                        sz[z][p][0:H, :].bitcast(mybir.dt.uint16 if MDT == BF16
                                                 else mybir.dt.uint32), 0)
                    nc.sync.dma_start(sz[z][p][K - 1:K, :], ones_d[0:1, hb])
                    nc.sync.dma_start(sz[z][p][H:H + IN, :], x_d[p, :, hb])
                nc.vector.memset(cz[z][:], 0.0)

            def emit_P1(z, t):
                """Gate matmuls (order f,o,g,i) + x prefetch for t+2."""
                W = encw if t < T else (dec0w if t == T else decfw)
                prev = sz[z][t % 2]
                pt = ptz[z]
                for gi in range(4):
                    nc.tensor.matmul(pt[:, gi * HB:(gi + 1) * HB],
                                     W[:, gi * H:(gi + 1) * H],
                                     prev[:], start=True, stop=True)
                if t + 2 < T:
                    hb = slice(z * HB, (z + 1) * HB)
                    nc.sync.dma_start(sz[z][t % 2][H:H + IN, :],
                                      x_d[t + 2, :, hb])
                for _ in range(WARM):
                    nc.tensor.matmul(junkp[:], W[:, 0:H], W[:, 0:4 * H],
                                     start=True, stop=True)
                return pt

            def emit_sigma(z, pt):
                S = work.tile([H, 4 * HB], EWDT, tag=f"S{z}", name=f"S{z}")
                if SPLIT_SIGMA:
                    nc.scalar.activation(S[:, 0:2 * HB], pt[:, 0:2 * HB],
                                         AF.Sigmoid)
                    nc.scalar.activation(S[:, 2 * HB:4 * HB],
                                         pt[:, 2 * HB:4 * HB], AF.Sigmoid)
                else:
                    nc.scalar.activation(S[:], pt[:], AF.Sigmoid)
                return S

            def emit_P2(z, t, pt, S):
                """Cell/hidden tail for chain z, step t.

                S blocks: [f | o | 2g | i]."""
                c = cz[z]
                Sf, So = S[:, 0:HB], S[:, HB:2 * HB]
                Sg, Si = S[:, 2 * HB:3 * HB], S[:, 3 * HB:4 * HB]
                w = work.tile([H, HB], CDT, tag=f"w{z}", name=f"w{z}")
                w_eng.tensor_mul(w[:], Sf, c[:])
                v = work.tile([H, HB], EWDT, tag=f"v{z}", name=f"v{z}")
                v_eng.scalar_tensor_tensor(v[:], Sg, -0.5, Si,
                                           ALU.add, ALU.mult)
                nc.vector.tensor_add(c[:], v[:], w[:])
                nxt = sz[z][(t + 1) % 2]
                import contextlib
                hp = tc.high_priority() if HIPRI_H else contextlib.nullcontext()
                with hp:
                    nc.vector._custom_dve(
                        _TANH2_MUL, out=nxt[0:H, :], in0=c[:], in1=So,
                        s0=float(TA), s1=float(TB), imm2=float(TC))
                hb = slice(z * HB, (z + 1) * HB)
                if t == T - 1:
                    nc.vector.memset(c[:], 0.0)
                elif t >= T:
                    d = t - T
                    yp = pt[0:OUT, 0:HB]
                    nc.tensor.matmul(yp, wy[:], nxt[:], start=True, stop=True)
                    yo = work.tile([OUT, HB], F32, tag=f"yo{z}", name=f"yo{z}")
                    nc.vector.tensor_copy(yo[:], yp)
                    nc.sync.dma_start(y_d[d, :, hb], yo[:])

            # Software pipeline: chain z's P1/sigma interleaves with the
            # previous chain's pending tail, anti-phasing the chains on
            # the in-order engines.
            pend = {}
            for t in range(T + DEC):
                for z in range(nh):
                    pt = emit_P1(z, t)
                    zo = (z + 1) % nh
                    if zo in pend:
                        emit_P2(**pend.pop(zo))
                    S = emit_sigma(z, pt)
                    pend[z] = dict(z=z, t=t, pt=pt, S=S)
            for z in list(pend):
                emit_P2(**pend.pop(z))
    nc.finalize()
    return nc


def kernel(inputs, W_ih_enc, W_hh_enc, b_ih_enc, b_hh_enc,
           W_ih_dec, W_hh_dec, b_ih_dec, b_hh_dec, W_y, b_y,
           _trace=False, _perf_out=None):
    f64 = np.float64
    encw = _pack_weights(np.asarray(W_hh_enc, f64), np.asarray(W_ih_enc, f64),
                         np.asarray(b_ih_enc, f64) + np.asarray(b_hh_enc, f64))
    Wihd = np.asarray(W_ih_dec, f64)
    Whhd = np.asarray(W_hh_dec, f64)
    bd = np.asarray(b_ih_dec, f64) + np.asarray(b_hh_dec, f64)
    Wyf = np.asarray(W_y, f64)
    byf = np.asarray(b_y, f64)
    dec0w = _pack_weights(Whhd, None, bd)
    decfw = _pack_weights(Whhd + Wihd @ Wyf, None, bd + Wihd @ byf)
    ndt = _np_dt(MDT)
    wyk = np.zeros((K, OUT), np.float64)
    wyk[0:H, :] = TK * Wyf.T
    wyk[K - 1, :] = byf
    wyk = wyk.astype(ndt)
    ones = np.ones((1, BS), ndt)

    nc = _build_program()

    inputs = np.asarray(inputs, np.float32)
    in_maps = []
    for core in range(NCORES):
        xs = inputs[:, core * BS:(core + 1) * BS, :]         # [T, BS, IN]
        xt = np.ascontiguousarray(xs.transpose(0, 2, 1)).astype(ndt)
        in_maps.append({"x": xt, "encw": encw, "dec0w": dec0w,
                        "decfw": decfw, "wy": wyk, "ones": ones})

    import time as _time
    res = run_bass_kernel_spmd(nc, in_maps, core_ids=list(range(NCORES)),
                               trace=_trace)
    if _perf_out is not None:
        walls = []
        for _ in range(6):
            t0 = _time.time()
            res = run_bass_kernel_spmd(nc, in_maps,
                                       core_ids=list(range(NCORES)),
                                       trace=_trace)
            walls.append(int((_time.time() - t0) * 1e9))
        _perf_out.update(exec_time_ns=res.exec_time_ns, walls_ns=walls,
                         trace=res.instructions_and_trace,
                         profile_json=res.profile_json)
    out = np.empty((DEC, B, OUT), np.float32)
    for core in range(NCORES):
        y = res.results[core]["y"]                           # [DEC, OUT, BS]
        out[:, core * BS:(core + 1) * BS, :] = y.transpose(0, 2, 1)
    return out
